# revision 21
# baseline (speedup 1.0000x reference)
"""Expert-parallel MoE FFN kernel for Trainium2 (8 NeuronCores).

Reference computation (per expert e):
    y[:, e*C:(e+1)*C, :] = gelu(x_e @ w1[e] + b1[e]) @ w2[e] + b2[e]

Sharding: expert-parallel - core e owns expert e (E == n_cores == 8) and the
matching chunk of dim 1 of `inputs`. No cross-core communication.

Per-core dataflow (T=16384 tokens, D=512, F=2048), all matmuls bf16:
  - X is pre-transposed and pre-cast to bf16 on the HOST into chunk-major
    [NCHUNK, 128d, DSUB, 512t] layout, so the device just streams one linear
    512KB DMA per 512-token chunk straight into SBUF. No on-device cast,
    no DRAM bounce, no XBAR transpose -> the TensorE stream is pure matmuls
    and never starves (the old transpose chain caused ~50us of PE idle and
    5 HAM cold/warm oscillations in the first 220us).
  - mm1: H^T[f, t] += W1[d, f].T @ X^T[d, t]; gelu+b1 fused on ScalarE
    (f on partitions -> b1 is a per-partition bias), H stored bf16.
  - mm2: Y[t, d] += (H^T[f, t128]).T @ W2[f, d] with H^T as the stationary
    operand, so Y comes out token-major and stores contiguously.
  - Weights live in SBUF whole-kernel; staged as 16 linear 128KB DMAs each
    (fs-block order) on the scalar queue so mm1's first f-tiles land early.

PE roofline for this shape at bf16: 4096 matmuls x ~216ns (N=512) ~= 888us.
"""

import numpy as np
import ml_dtypes

import concourse.bacc as bacc
import concourse.bass as bass
import concourse.mybir as mybir
import concourse.tile as tile
from concourse.bass_utils import run_bass_kernel_spmd

B, EC, D = 16, 8192, 512
E, F = 8, 2048
C = EC // E            # capacity per expert = 1024
T = B * C              # tokens per expert/core = 16384
P = 128
DSUB = D // P          # 4
FSUB = F // P          # 16
TCHUNK = 512
TS = TCHUNK // P       # 4
NCHUNK = T // TCHUNK   # 32
NTILE = T // P         # 128
N_CORES = 8

# Stash of the last BassKernelResults (for test harness profiling).
LAST_RESULT = None


def build_nc(n_tokens: int = T, act_func=None):
    if act_func is None:
        act_func = mybir.ActivationFunctionType.Gelu_apprx_tanh
    nchunk = n_tokens // TCHUNK
    nc = bacc.Bacc(
        "TRN2",
        target_bir_lowering=False,
        debug=False,
        num_devices=N_CORES,
    )
    # x pre-transposed on host: x[c, dp, ds*TCHUNK + s] = X[c*TCHUNK+s, ds*128+dp]
    x = nc.dram_tensor(
        "x", [nchunk, P, DSUB * TCHUNK], mybir.dt.bfloat16, kind="ExternalInput"
    ).ap()
    # w1[dp, fs, ds*128 + j] = W1[ds*128+dp, fs*128+j]  (partition-major,
    # same layout as the SBUF tile -> block DMAs with 4-5KB runs/partition)
    w1 = nc.dram_tensor(
        "w1", [P, FSUB, DSUB * P], mybir.dt.bfloat16, kind="ExternalInput"
    ).ap()
    b1 = nc.dram_tensor("b1", [P, FSUB], mybir.dt.float32, kind="ExternalInput").ap()
    # w2[fp, fs, d] = W2[fs*128+fp, d]
    w2 = nc.dram_tensor(
        "w2", [P, FSUB, D], mybir.dt.bfloat16, kind="ExternalInput"
    ).ap()
    b2 = nc.dram_tensor("b2", [P, D], mybir.dt.float32, kind="ExternalInput").ap()
    # y[i, tp, d] = Y[i*128+tp, d]
    y = nc.dram_tensor(
        "y", [n_tokens // P, P, D], mybir.dt.float32, kind="ExternalOutput"
    ).ap()

    with tile.TileContext(nc) as tc:
        with (
            tc.tile_pool(name="consts", bufs=1) as consts,
            tc.tile_pool(name="xt", bufs=3) as xt_pool,
            tc.tile_pool(name="h", bufs=3) as h_pool,
            tc.tile_pool(name="yout", bufs=4) as y_pool,
            tc.tile_pool(name="ps_h", bufs=4, space="PSUM") as ps_h,
            tc.tile_pool(name="ps_y", bufs=3, space="PSUM") as ps_y,
            tc.tile_pool(name="ps_w", bufs=1, space="PSUM") as ps_w,
        ):
            # Weight staging: one queue (scalar), in exact consumption order,
            # as few-but-big block DMAs (parallel queues just steal DMA-engine
            # bandwidth from the critical path; per-fs DMAs make 1KB packets
            # that drain ~4x slower than 4KB+ ones). The x stream owns sync.
            b1_sb = consts.tile([P, FSUB], mybir.dt.float32)
            nc.gpsimd.dma_start(b1_sb, b1)
            b2_sb = consts.tile([P, D], mybir.dt.float32)
            nc.gpsimd.dma_start(b2_sb, b2)
            w1_sb = consts.tile([P, FSUB, DSUB * P], mybir.dt.bfloat16)
            for lo, hi in ((0, 2), (2, 4), (4, 8), (8, 12), (12, 16)):
                nc.scalar.dma_start(w1_sb[:, lo:hi, :], w1[:, lo:hi, :])
            w2_sb = consts.tile([P, FSUB, D], mybir.dt.bfloat16)
            for lo in range(0, FSUB, 4):
                nc.scalar.dma_start(w2_sb[:, lo:lo + 4, :], w2[:, lo:lo + 4, :])

            # HAM warmup: the PE boots idle at 1.2GHz and only reaches 2.4GHz
            # after ~3.4us of sustained activity. Real matmuls can't start
            # until weights/x land (~13us), but the engines are booted by
            # ~6.5us -- so burn the DMA wait on dummy matmuls over scratch
            # (never-written) tiles. The real stream then starts warm.
            warm_l = consts.tile([P, P], mybir.dt.bfloat16)
            warm_r = consts.tile([P, TCHUNK], mybir.dt.bfloat16)
            nc.vector.memset(warm_l, 0.0)
            nc.vector.memset(warm_r, 0.0)
            pwarm = ps_w.tile([P, TCHUNK], mybir.dt.float32)
            for i in range(12):
                nc.tensor.matmul(
                    pwarm, lhsT=warm_l, rhs=warm_r, start=(i == 0), stop=(i == 11)
                )

            h_tiles = {}

            def do_mm1(c):
                # One linear 512KB DMA per chunk: X^T bf16, token-chunk major.
                xt = xt_pool.tile([P, DSUB * TCHUNK], mybir.dt.bfloat16)
                nc.sync.dma_start(xt, x[c])
                # mm1 + fused gelu/bias: H^T[f, t] bf16.
                h = h_pool.tile([P, FSUB, TCHUNK], mybir.dt.bfloat16)
                h_tiles[c] = h
                for fs in range(FSUB):
                    ph = ps_h.tile([P, TCHUNK], mybir.dt.float32)
                    for ds in range(DSUB):
                        nc.tensor.matmul(
                            ph,
                            lhsT=w1_sb[:, fs, ds * P:(ds + 1) * P],
                            rhs=xt[:, ds * TCHUNK:(ds + 1) * TCHUNK],
                            start=(ds == 0),
                            stop=(ds == DSUB - 1),
                        )
                    nc.scalar.activation(
                        h[:, fs, :],
                        ph,
                        act_func,
                        bias=b1_sb[:, fs:fs + 1],
                        scale=1.0,
                    )

            def do_mm2(c):
                # mm2: Y[t, d] per 128-token subtile; + b2; store.
                h = h_tiles.pop(c)
                for ts in range(TS):
                    py = ps_y.tile([P, D], mybir.dt.float32)
                    for fs in range(FSUB):
                        nc.tensor.matmul(
                            py,
                            lhsT=h[:, fs, ts * P:(ts + 1) * P],
                            rhs=w2_sb[:, fs, :],
                            start=(fs == 0),
                            stop=(fs == FSUB - 1),
                        )
                    y_sb = y_pool.tile([P, D], mybir.dt.float32)
                    nc.vector.tensor_add(y_sb, py, b2_sb)
                    # y stores issue from the (otherwise idle) gpsimd queue:
                    # on the scalar queue their ~0.6us issue cost pushes
                    # ACTIVATEs late (ScalarE is ~98% busy in mm1 phases) and
                    # the resulting ps_h backpressure stalls the PE.
                    nc.gpsimd.dma_start(y[c * TS + ts], y_sb)

            # Software-pipeline mm2 one chunk behind mm1: chunk 0's mm2 then
            # issues ~14us later, fully hiding the w2 staging DMAs with zero
            # PE idle (otherwise mm2(c0) stalls ~5us waiting for w2).
            do_mm1(0)
            for c in range(1, nchunk):
                do_mm1(c)
                do_mm2(c - 1)
            do_mm2(nchunk - 1)

    nc.compile()
    return nc


_NC_CACHE = {}


def _get_nc(n_tokens: int = T):
    if n_tokens not in _NC_CACHE:
        _NC_CACHE[n_tokens] = build_nc(n_tokens)
    return _NC_CACHE[n_tokens]


def make_in_maps(inputs, w1, b1, w2, b2):
    """Shard + lay out host-side: core e gets expert e."""
    bf16 = ml_dtypes.bfloat16
    inputs = np.asarray(inputs)
    w1, b1 = np.asarray(w1), np.asarray(b1)
    w2, b2 = np.asarray(w2), np.asarray(b2)
    in_maps = []
    for e in range(E):
        # X^T chunk-major bf16: [NCHUNK, P, DSUB*TCHUNK]
        x_bf = np.asarray(inputs[:, e * C:(e + 1) * C, :], dtype=bf16)
        x_e = np.ascontiguousarray(
            x_bf.reshape(NCHUNK, TCHUNK, DSUB, P).transpose(0, 3, 2, 1)
        ).reshape(NCHUNK, P, DSUB * TCHUNK)
        # w1[e] [D, F] -> [P, FSUB, DSUB*128]: [dp, fs, ds*128+j]
        w1_e = np.ascontiguousarray(
            w1[e].astype(bf16).reshape(DSUB, P, FSUB, P).transpose(1, 2, 0, 3)
        ).reshape(P, FSUB, DSUB * P)
        # b1[e] [F] -> [P, FSUB] with f = fs*128 + p
        b1_e = np.ascontiguousarray(
            b1[e].reshape(FSUB, P).T.astype(np.float32)
        )
        # w2[e] [F, D] -> [P, FSUB, D] with f = fs*128 + p
        w2_e = np.ascontiguousarray(
            w2[e].astype(bf16).reshape(FSUB, P, D).transpose(1, 0, 2)
        )
        # b2[e] [D] -> broadcast to [P, D]
        b2_e = np.ascontiguousarray(
            np.broadcast_to(b2[e].astype(np.float32), (P, D))
        )
        in_maps.append(
            {"x": x_e, "w1": w1_e, "b1": b1_e, "w2": w2_e, "b2": b2_e}
        )
    return in_maps


def kernel(inputs, w1, b1, w2, b2):
    global LAST_RESULT
    nc = _get_nc(T)
    in_maps = make_in_maps(inputs, w1, b1, w2, b2)
    res = run_bass_kernel_spmd(nc, in_maps, core_ids=list(range(N_CORES)))
    LAST_RESULT = res
    out = np.empty((B, EC, D), dtype=np.float32)
    for e in range(E):
        out[:, e * C:(e + 1) * C, :] = np.asarray(res.results[e]["y"]).reshape(
            B, C, D
        )
    return out
